# revision 26
# baseline (speedup 1.0000x reference)
"""DeepSeek-V3-style MoE layer on 8 Trainium2 NeuronCores.

Strategy (uniform expert-parallel, shared expert folded into routed path):
  - Router (sigmoid over rand_logits, top-4, capacity drop) runs on host:
    it is O(T*E) index math that determines the dispatch, i.e. the sharding.
  - The shared expert (MS = 2816 = 2 x 1408) is exactly two standard-shaped
    experts (D -> M SwiGLU -> D). Each half is token-split 4 ways: cores 0-3
    run half 0, cores 4-7 run half 1, each over a 512-token quarter. This
    removes the 352->384 intermediate padding the sliced layout needed.
  - The 32 routed experts are placed one per (core, segment) cell on a
    4-segment grid; segment capacities are the max routed load in each
    sorted octile (SPMD: every core runs the identical instruction stream).
  - y is written back as [d-tile, 128, tok] fp16 (no on-chip transpose);
    the host transposes, applies routing weights, and scatter-adds.

All matmuls run on the tensor engine with fp16 operands (fp32 PSUM).
"""

import functools
import os
import sys
import time

import numpy as np

for _p in ('/opt/trn_rl_repo', '/root/.axon_site/_ro/trn_rl_repo'):
    if os.path.isdir(_p) and _p not in sys.path:
        sys.path.insert(0, _p)

import concourse.bass as bass  # noqa: F401  (AP helpers)
import concourse.tile as tile
from concourse import bacc, mybir
from concourse.bass_utils import run_bass_kernel_spmd

# ---- problem config (hardcoded from spec) ----
T = 2048
D = 2048          # hidden
M = 1408          # expert intermediate
E = 32            # experts
K = 4             # top_k
CAP = 512         # per-expert capacity
ROUTE_SCALE = 2.5
N_CORES = 8
NSEG = 5          # 1 shared-half segment + 4 routed segments
KT = D // 128     # 16 contraction tiles over hidden
MT = M // 128     # 11 intermediate tiles
SH_CAP = T // 4   # 512 tokens per shared-half quarter

DT, NP_DT = mybir.dt.float16, np.float16
F32 = mybir.dt.float32
SILU = mybir.ActivationFunctionType.Silu


# --------------------------------------------------------------------------
# host-side routing
# --------------------------------------------------------------------------

def _route(rand_logits, expert_bias):
    scores = (1.0 / (1.0 + np.exp(-rand_logits.astype(np.float32)))).astype(np.float32)
    biased = scores + expert_bias[None, :]
    idx = np.argsort(-biased, axis=1, kind="stable")[:, :K]          # [T, K]
    top = np.take_along_axis(scores, idx, axis=1)
    top = top / (top.sum(-1, keepdims=True) + 1e-20) * ROUTE_SCALE   # [T, K]

    flat_e = idx.reshape(-1)
    order = np.argsort(flat_e, kind="stable")                        # assignment ids by expert
    counts = np.bincount(flat_e, minlength=E)
    kept = np.minimum(counts, CAP)
    starts = np.concatenate([[0], np.cumsum(counts)])[:E]
    assigns = [order[starts[e]: starts[e] + kept[e]] for e in range(E)]
    return top, assigns, kept


def _placement(kept):
    """Experts -> (segment, core) grid; segment cap = max load in its octile."""
    rank = np.argsort(-kept, kind="stable")
    slots = rank.reshape(4, N_CORES)                 # routed segment s, core c
    caps = (SH_CAP,) + tuple(int(kept[slots[s][0]]) for s in range(4))
    return slots, caps


# --------------------------------------------------------------------------
# device program
# --------------------------------------------------------------------------

# slot = group of segments sharing one ht/y tensor (keeps DMA runs >= 512B)
SLOT_SEGS = ([0], [1, 2], [3, 4])


@functools.lru_cache(maxsize=4)
def _program(caps):
    capsum = sum(caps)
    offs = [0]
    for c in caps:
        offs.append(offs[-1] + c)

    nc = bacc.Bacc("TRN2", target_bir_lowering=False, debug=False,
                   num_devices=N_CORES)
    ap = {}
    ap["xt"] = nc.dram_tensor("xt", [KT, 128, capsum], DT, kind="ExternalInput").ap()
    ap["wg"] = nc.dram_tensor("wg", [NSEG, MT, 128, KT * 128], DT, kind="ExternalInput").ap()
    ap["wu"] = nc.dram_tensor("wu", [NSEG, MT, 128, KT * 128], DT, kind="ExternalInput").ap()
    ap["wd"] = nc.dram_tensor("wd", [NSEG, MT, 128, D], DT, kind="ExternalInput").ap()
    for si, segs in enumerate(SLOT_SEGS):
        w = sum(caps[s] for s in segs)
        ap[f"yr{si}"] = nc.dram_tensor(f"yr{si}", [KT, 128, w], DT,
                                       kind="ExternalOutput").ap()


    with tile.TileContext(nc) as tc:
        with tc.tile_pool(name="xtp", bufs=1) as xtp, \
             tc.tile_pool(name="wp", bufs=8) as wp, \
             tc.tile_pool(name="hp", bufs=2) as hp, \
             tc.tile_pool(name="wdp", bufs=6) as wdp, \
             tc.tile_pool(name="actp", bufs=3) as actp, \
             tc.tile_pool(name="obp", bufs=4) as obp, \
             tc.tile_pool(name="psgu", bufs=6, space="PSUM") as psgu, \
             tc.tile_pool(name="psy", bufs=2, space="PSUM") as psy:

            xt_sb = xtp.tile([128, KT, capsum], DT, name="xt_sb")

            for si, segs in enumerate(SLOT_SEGS):
                soff = offs[segs[0]]                      # global col offset
                scap = sum(caps[s] for s in segs)
                # local (offset, cap) of each segment within the slot
                lseg = []
                o = 0
                for s in segs:
                    lseg.append((s, o, caps[s]))
                    o += caps[s]

                ht = hp.tile([128, MT, scap], DT, name="ht", tag="ht")

                def gu_mm(psg, psu, wg_sb, wu_sb, s, c0, c1):
                    rhs = xt_sb[:, :, offs[s] + c0: offs[s] + c1]
                    for t in range(KT):
                        nc.tensor.matmul(psg[:], wg_sb[:, t * 128:(t + 1) * 128],
                                         rhs[:, t, :], start=(t == 0), stop=(t == KT - 1))
                        nc.tensor.matmul(psu[:], wu_sb[:, t * 128:(t + 1) * 128],
                                         rhs[:, t, :], start=(t == 0), stop=(t == KT - 1))

                def act_mul(psg, psu, c, m, lo):
                    sact = actp.tile([128, c], F32, name="sact", tag="act")
                    nc.scalar.activation(sact[:], psg[:], SILU)
                    nc.vector.tensor_mul(ht[:, m, lo:lo + c], sact[:], psu[:])

                def xtld(t0, t1, c0, c1):
                    nc.sync.dma_start(
                        xt_sb[:, t0:t1, c0:c1],
                        ap["xt"][t0:t1].transpose([1, 0, 2])[:, :, c0:c1])

                for m in range(MT):
                    wpairs = []
                    for s, lo, c in lseg:
                        wg_sb = wp.tile([128, KT * 128], DT, name="wg_sb", tag="w")
                        wu_sb = wp.tile([128, KT * 128], DT, name="wu_sb", tag="w")
                        if si == 0 and m == 0:
                            # cold start: need-ordered chunks; token tiles as
                            # pairs to dodge the per-DMA descriptor cadence
                            # while feeding the k-loop
                            nc.sync.dma_start(wg_sb[:, :512], ap["wg"][s, m, :, :512])
                            xtld(0, 2, 0, SH_CAP)
                            nc.sync.dma_start(wu_sb[:, :512], ap["wu"][s, m, :, :512])
                            xtld(2, 4, 0, SH_CAP)
                            nc.sync.dma_start(wg_sb[:, 512:1024], ap["wg"][s, m, :, 512:1024])
                            nc.sync.dma_start(wu_sb[:, 512:1024], ap["wu"][s, m, :, 512:1024])
                            xtld(4, 6, 0, SH_CAP)
                            nc.sync.dma_start(wg_sb[:, 1024:], ap["wg"][s, m, :, 1024:])
                            nc.sync.dma_start(wu_sb[:, 1024:], ap["wu"][s, m, :, 1024:])
                            xtld(6, 8, 0, SH_CAP)
                            xtld(8, 10, 0, SH_CAP)
                            xtld(10, 12, 0, SH_CAP)
                            xtld(12, 14, 0, SH_CAP)
                            xtld(14, 16, 0, SH_CAP)
                        else:
                            # stream gate/up halves interleaved so the k-loop
                            # can start before the full m-tile lands
                            nc.sync.dma_start(wg_sb[:, :1024], ap["wg"][s, m, :, :1024])
                            nc.sync.dma_start(wu_sb[:, :1024], ap["wu"][s, m, :, :1024])
                            nc.sync.dma_start(wg_sb[:, 1024:], ap["wg"][s, m, :, 1024:])
                            nc.sync.dma_start(wu_sb[:, 1024:], ap["wu"][s, m, :, 1024:])
                        wpairs.append((wg_sb, wu_sb))
                    if si == 0 and 2 <= m < 10:
                        # backfill routed token columns (needed from slot 1 on)
                        for t in range(2 * (m - 2), 2 * (m - 1)):
                            nc.sync.dma_start(xt_sb[:, t, SH_CAP:],
                                              ap["xt"][t][:, SH_CAP:])

                    for (s, lo, c), (wg_sb, wu_sb) in zip(lseg, wpairs):
                        psg = psgu.tile([128, c], F32, name="psg", tag="psgu")
                        psu = psgu.tile([128, c], F32, name="psu", tag="psgu")
                        gu_mm(psg, psu, wg_sb, wu_sb, s, 0, c)
                        act_mul(psg, psu, c, m, lo)

                # down-projection: out stays [d-part, tok]; host re-layouts
                for g in range(4):
                    wds = []
                    for s, lo, c in lseg:
                        wd_g = wdp.tile([128, MT, 512], DT, name="wd_g", tag="wd")
                        nc.sync.dma_start(
                            wd_g[:],
                            ap["wd"][s].transpose([1, 0, 2])[:, :, g * 512:(g + 1) * 512])
                        wds.append(wd_g)
                    for k in range(4):
                        ob = obp.tile([128, scap], DT, name="ob", tag="ob")
                        for (s, lo, c), wd_g in zip(lseg, wds):
                            ps = psy.tile([128, c], F32, name="ps_y", tag="psy")
                            for m in range(MT):
                                nc.tensor.matmul(ps[:], wd_g[:, m, k * 128:(k + 1) * 128],
                                                 ht[:, m, lo:lo + c],
                                                 start=(m == 0), stop=(m == MT - 1))
                            nc.vector.tensor_copy(ob[:, lo:lo + c], ps[:])
                        nc.sync.dma_start(ap[f"yr{si}"][g * 4 + k], ob[:])
    nc.compile()
    return nc


# --------------------------------------------------------------------------
# host-side packing + combine
# --------------------------------------------------------------------------

def _pack_gu(w):
    # [D, M] -> [MT, 128(k-part), KT*128] stationary-ready layout
    return np.ascontiguousarray(
        w.reshape(KT, 128, MT, 128).transpose(2, 1, 0, 3).reshape(MT, 128, KT * 128))


def kernel(**inputs):
    x = np.asarray(inputs["x"], np.float32)
    rand_logits = np.asarray(inputs["rand_logits"], np.float32)
    expert_bias = np.asarray(inputs["expert_bias"], np.float32)
    wg = np.asarray(inputs["w_gate"], np.float32)
    wu = np.asarray(inputs["w_up"], np.float32)
    wd = np.asarray(inputs["w_down"], np.float32)
    swg = np.asarray(inputs["sw_gate"], np.float32)
    swu = np.asarray(inputs["sw_up"], np.float32)
    swd = np.asarray(inputs["sw_down"], np.float32)

    top, assigns, kept = _route(rand_logits, expert_bias)
    slots, caps = _placement(kept)
    capsum = sum(caps)
    offs = np.concatenate([[0], np.cumsum(caps)]).astype(int)

    global _last_caps
    _last_caps = caps
    t0 = time.time()
    nc = _program(caps)
    t1 = time.time()

    xT = np.ascontiguousarray(x.T.astype(NP_DT))                    # [D, T]

    in_maps = []
    for c in range(N_CORES):
        half, quarter = c // 4, c % 4
        xt = np.zeros((D, capsum), NP_DT)
        xt[:, :SH_CAP] = xT[:, quarter * SH_CAP:(quarter + 1) * SH_CAP]
        for s in range(4):
            e = slots[s][c]
            tok = assigns[e] // K
            if len(tok):
                xt[:, offs[s + 1]: offs[s + 1] + len(tok)] = xT[:, tok]

        seg_w = [(swg[:, half * M:(half + 1) * M],
                  swu[:, half * M:(half + 1) * M],
                  swd[half * M:(half + 1) * M, :])]
        seg_w += [(wg[slots[s][c]], wu[slots[s][c]], wd[slots[s][c]])
                  for s in range(4)]
        wgx = np.stack([_pack_gu(g) for g, _, _ in seg_w])
        wux = np.stack([_pack_gu(u) for _, u, _ in seg_w])
        wdx = np.stack([d.reshape(MT, 128, D) for _, _, d in seg_w])

        in_maps.append({
            "xt": xt.reshape(KT, 128, capsum),
            "wg": wgx.astype(NP_DT),
            "wu": wux.astype(NP_DT),
            "wd": wdx.astype(NP_DT),
        })

    t2 = time.time()
    res = run_bass_kernel_spmd(nc, in_maps, core_ids=list(range(N_CORES)))
    t3 = time.time()
    if os.environ.get("BASSMOE_VERBOSE"):
        print(f"[kernel] program build {t1 - t0:.2f}s  pack {t2 - t1:.2f}s  "
              f"device run {t3 - t2:.2f}s", file=sys.stderr)
    outs = res.results

    # slot tensors -> per-core [tok, D] blocks
    def unpack(arr):                                    # [KT, 128, w] -> [w, D]
        return arr.transpose(2, 0, 1).reshape(arr.shape[2], D).astype(np.float32)

    out = np.zeros((T, D), np.float32)
    ytk = np.zeros((T, K, D), np.float32)
    for c in range(N_CORES):
        quarter = c % 4
        ysh = unpack(outs[c]["yr0"])                    # shared-half quarter
        out[quarter * SH_CAP:(quarter + 1) * SH_CAP] += ysh
        y1 = unpack(outs[c]["yr1"])
        y2 = unpack(outs[c]["yr2"])
        lofs = (0, 0, caps[1], 0, caps[3])
        ys = (None, y1, y1, y2, y2)
        for s in range(4):
            e = slots[s][c]
            a = assigns[e]
            if len(a):
                ytk[a // K, a % K] = ys[s + 1][lofs[s + 1]: lofs[s + 1] + len(a)]
    out += (top[:, :, None].astype(np.float32) * ytk).sum(axis=1)
    return out.astype(np.float32)


# revision 30
# speedup vs baseline: 1.0007x; 1.0007x over previous
"""DeepSeek-V3-style MoE layer on 8 Trainium2 NeuronCores.

Strategy (uniform expert-parallel, shared expert folded into routed path):
  - Router (sigmoid over rand_logits, top-4, capacity drop) runs on host:
    it is O(T*E) index math that determines the dispatch, i.e. the sharding.
  - The shared expert (MS = 2816 = 2 x 1408) is exactly two standard-shaped
    experts (D -> M SwiGLU -> D). Each half is token-split 4 ways: cores 0-3
    run half 0, cores 4-7 run half 1, each over a 512-token quarter. This
    removes the 352->384 intermediate padding the sliced layout needed.
  - The 32 routed experts are placed one per (core, segment) cell on a
    4-segment grid; segment capacities are the max routed load in each
    sorted octile (SPMD: every core runs the identical instruction stream).
  - y is written back as [d-tile, 128, tok] fp16 (no on-chip transpose);
    the host transposes, applies routing weights, and scatter-adds.

All matmuls run on the tensor engine with fp16 operands (fp32 PSUM).
"""

import functools
import os
import sys
import time

import numpy as np

for _p in ('/opt/trn_rl_repo', '/root/.axon_site/_ro/trn_rl_repo'):
    if os.path.isdir(_p) and _p not in sys.path:
        sys.path.insert(0, _p)

import concourse.bass as bass  # noqa: F401  (AP helpers)
import concourse.tile as tile
from concourse import bacc, mybir
from concourse.bass_utils import run_bass_kernel_spmd

# ---- problem config (hardcoded from spec) ----
T = 2048
D = 2048          # hidden
M = 1408          # expert intermediate
E = 32            # experts
K = 4             # top_k
CAP = 512         # per-expert capacity
ROUTE_SCALE = 2.5
N_CORES = 8
NSEG = 5          # 1 shared-half segment + 4 routed segments
KT = D // 128     # 16 contraction tiles over hidden
MT = M // 128     # 11 intermediate tiles
SH_CAP = T // 4   # 512 tokens per shared-half quarter

DT, NP_DT = mybir.dt.float16, np.float16
F32 = mybir.dt.float32
SILU = mybir.ActivationFunctionType.Silu


# --------------------------------------------------------------------------
# host-side routing
# --------------------------------------------------------------------------

def _route(rand_logits, expert_bias):
    scores = (1.0 / (1.0 + np.exp(-rand_logits.astype(np.float32)))).astype(np.float32)
    biased = scores + expert_bias[None, :]
    idx = np.argsort(-biased, axis=1, kind="stable")[:, :K]          # [T, K]
    top = np.take_along_axis(scores, idx, axis=1)
    top = top / (top.sum(-1, keepdims=True) + 1e-20) * ROUTE_SCALE   # [T, K]

    flat_e = idx.reshape(-1)
    order = np.argsort(flat_e, kind="stable")                        # assignment ids by expert
    counts = np.bincount(flat_e, minlength=E)
    kept = np.minimum(counts, CAP)
    starts = np.concatenate([[0], np.cumsum(counts)])[:E]
    assigns = [order[starts[e]: starts[e] + kept[e]] for e in range(E)]
    return top, assigns, kept


def _placement(kept):
    """Experts -> (segment, core) grid; segment cap = max load in its octile."""
    rank = np.argsort(-kept, kind="stable")
    slots = rank.reshape(4, N_CORES)                 # routed segment s, core c
    caps = (SH_CAP,) + tuple(int(kept[slots[s][0]]) for s in range(4))
    return slots, caps


# --------------------------------------------------------------------------
# device program
# --------------------------------------------------------------------------

# slot = group of segments sharing one ht/y tensor (keeps DMA runs >= 512B)
SLOT_SEGS = ([0], [1, 2], [3, 4])


@functools.lru_cache(maxsize=4)
def _program(caps):
    capsum = sum(caps)
    offs = [0]
    for c in caps:
        offs.append(offs[-1] + c)

    nc = bacc.Bacc("TRN2", target_bir_lowering=False, debug=False,
                   num_devices=N_CORES)
    ap = {}
    ap["xt"] = nc.dram_tensor("xt", [KT, 128, capsum], DT, kind="ExternalInput").ap()
    ap["wg"] = nc.dram_tensor("wg", [NSEG, MT, 128, KT * 128], DT, kind="ExternalInput").ap()
    ap["wu"] = nc.dram_tensor("wu", [NSEG, MT, 128, KT * 128], DT, kind="ExternalInput").ap()
    ap["wd"] = nc.dram_tensor("wd", [NSEG, MT, 128, D], DT, kind="ExternalInput").ap()
    for si, segs in enumerate(SLOT_SEGS):
        w = sum(caps[s] for s in segs)
        ap[f"yr{si}"] = nc.dram_tensor(f"yr{si}", [KT, 128, w], DT,
                                       kind="ExternalOutput").ap()


    with tile.TileContext(nc) as tc:
        with tc.tile_pool(name="xtp", bufs=1) as xtp, \
             tc.tile_pool(name="wp", bufs=8) as wp, \
             tc.tile_pool(name="hp", bufs=2) as hp, \
             tc.tile_pool(name="wdp", bufs=6) as wdp, \
             tc.tile_pool(name="actp", bufs=3) as actp, \
             tc.tile_pool(name="obp", bufs=4) as obp, \
             tc.tile_pool(name="psgu", bufs=6, space="PSUM") as psgu, \
             tc.tile_pool(name="psy", bufs=2, space="PSUM") as psy:

            xt_sb = xtp.tile([128, KT, capsum], DT, name="xt_sb")

            for si, segs in enumerate(SLOT_SEGS):
                soff = offs[segs[0]]                      # global col offset
                scap = sum(caps[s] for s in segs)
                # local (offset, cap) of each segment within the slot
                lseg = []
                o = 0
                for s in segs:
                    lseg.append((s, o, caps[s]))
                    o += caps[s]

                ht = hp.tile([128, MT, scap], DT, name="ht", tag="ht")

                def gu_mm(psg, psu, wg_sb, wu_sb, s, c0, c1):
                    rhs = xt_sb[:, :, offs[s] + c0: offs[s] + c1]
                    for t in range(KT):
                        nc.tensor.matmul(psg[:], wg_sb[:, t * 128:(t + 1) * 128],
                                         rhs[:, t, :], start=(t == 0), stop=(t == KT - 1))
                        nc.tensor.matmul(psu[:], wu_sb[:, t * 128:(t + 1) * 128],
                                         rhs[:, t, :], start=(t == 0), stop=(t == KT - 1))

                def act_mul(psg, psu, c, m, lo):
                    sact = actp.tile([128, c], F32, name="sact", tag="act")
                    nc.scalar.activation(sact[:], psg[:], SILU)
                    nc.vector.tensor_mul(ht[:, m, lo:lo + c], sact[:], psu[:])

                def xtld(t0, t1, c0, c1):
                    nc.sync.dma_start(
                        xt_sb[:, t0:t1, c0:c1],
                        ap["xt"][t0:t1].transpose([1, 0, 2])[:, :, c0:c1])

                for m in range(MT):
                    wpairs = []
                    for s, lo, c in lseg:
                        wg_sb = wp.tile([128, KT * 128], DT, name="wg_sb", tag="w")
                        wu_sb = wp.tile([128, KT * 128], DT, name="wu_sb", tag="w")
                        if si == 0 and m == 0:
                            # cold start: the head's critical path is the DMA
                            # engine itself, so every descriptor carries >=
                            # 625ns of transfer (the HWDGE cadence) — no dead
                            # time — and chunks are issued in need order
                            nc.sync.dma_start(wg_sb[:, :1024], ap["wg"][s, m, :, :1024])
                            xtld(0, 2, 0, SH_CAP)
                            nc.sync.dma_start(wu_sb[:, :1024], ap["wu"][s, m, :, :1024])
                            xtld(2, 4, 0, SH_CAP)
                            nc.sync.dma_start(wg_sb[:, 1024:], ap["wg"][s, m, :, 1024:])
                            nc.sync.dma_start(wu_sb[:, 1024:], ap["wu"][s, m, :, 1024:])
                            xtld(4, 6, 0, SH_CAP)
                            xtld(6, 8, 0, SH_CAP)
                            xtld(8, 10, 0, SH_CAP)
                            xtld(10, 12, 0, SH_CAP)
                            xtld(12, 14, 0, SH_CAP)
                            xtld(14, 16, 0, SH_CAP)
                        else:
                            # stream gate/up halves interleaved so the k-loop
                            # can start before the full m-tile lands
                            nc.sync.dma_start(wg_sb[:, :1024], ap["wg"][s, m, :, :1024])
                            nc.sync.dma_start(wu_sb[:, :1024], ap["wu"][s, m, :, :1024])
                            nc.sync.dma_start(wg_sb[:, 1024:], ap["wg"][s, m, :, 1024:])
                            nc.sync.dma_start(wu_sb[:, 1024:], ap["wu"][s, m, :, 1024:])
                        wpairs.append((wg_sb, wu_sb))
                    if si == 0 and 2 <= m < 10:
                        # backfill routed token columns (needed from slot 1 on)
                        for t in range(2 * (m - 2), 2 * (m - 1)):
                            nc.sync.dma_start(xt_sb[:, t, SH_CAP:],
                                              ap["xt"][t][:, SH_CAP:])

                    for (s, lo, c), (wg_sb, wu_sb) in zip(lseg, wpairs):
                        psg = psgu.tile([128, c], F32, name="psg", tag="psgu")
                        psu = psgu.tile([128, c], F32, name="psu", tag="psgu")
                        gu_mm(psg, psu, wg_sb, wu_sb, s, 0, c)
                        act_mul(psg, psu, c, m, lo)

                # down-projection: out stays [d-part, tok]; host re-layouts
                for g in range(4):
                    wds = []
                    for s, lo, c in lseg:
                        wd_g = wdp.tile([128, MT, 512], DT, name="wd_g", tag="wd")
                        nc.sync.dma_start(
                            wd_g[:],
                            ap["wd"][s].transpose([1, 0, 2])[:, :, g * 512:(g + 1) * 512])
                        wds.append(wd_g)
                    for k in range(4):
                        ob = obp.tile([128, scap], DT, name="ob", tag="ob")
                        for (s, lo, c), wd_g in zip(lseg, wds):
                            ps = psy.tile([128, c], F32, name="ps_y", tag="psy")
                            for m in range(MT):
                                nc.tensor.matmul(ps[:], wd_g[:, m, k * 128:(k + 1) * 128],
                                                 ht[:, m, lo:lo + c],
                                                 start=(m == 0), stop=(m == MT - 1))
                            nc.vector.tensor_copy(ob[:, lo:lo + c], ps[:])
                        nc.sync.dma_start(ap[f"yr{si}"][g * 4 + k], ob[:])
    nc.compile()
    return nc


# --------------------------------------------------------------------------
# host-side packing + combine
# --------------------------------------------------------------------------

def _pack_gu(w):
    # [D, M] -> [MT, 128(k-part), KT*128] stationary-ready layout
    return np.ascontiguousarray(
        w.reshape(KT, 128, MT, 128).transpose(2, 1, 0, 3).reshape(MT, 128, KT * 128))


def kernel(**inputs):
    x = np.asarray(inputs["x"], np.float32)
    rand_logits = np.asarray(inputs["rand_logits"], np.float32)
    expert_bias = np.asarray(inputs["expert_bias"], np.float32)
    wg = np.asarray(inputs["w_gate"], np.float32)
    wu = np.asarray(inputs["w_up"], np.float32)
    wd = np.asarray(inputs["w_down"], np.float32)
    swg = np.asarray(inputs["sw_gate"], np.float32)
    swu = np.asarray(inputs["sw_up"], np.float32)
    swd = np.asarray(inputs["sw_down"], np.float32)

    top, assigns, kept = _route(rand_logits, expert_bias)
    slots, caps = _placement(kept)
    capsum = sum(caps)
    offs = np.concatenate([[0], np.cumsum(caps)]).astype(int)

    global _last_caps
    _last_caps = caps
    t0 = time.time()
    nc = _program(caps)
    t1 = time.time()

    xT = np.ascontiguousarray(x.T.astype(NP_DT))                    # [D, T]

    in_maps = []
    for c in range(N_CORES):
        half, quarter = c // 4, c % 4
        xt = np.zeros((D, capsum), NP_DT)
        xt[:, :SH_CAP] = xT[:, quarter * SH_CAP:(quarter + 1) * SH_CAP]
        for s in range(4):
            e = slots[s][c]
            tok = assigns[e] // K
            if len(tok):
                xt[:, offs[s + 1]: offs[s + 1] + len(tok)] = xT[:, tok]

        seg_w = [(swg[:, half * M:(half + 1) * M],
                  swu[:, half * M:(half + 1) * M],
                  swd[half * M:(half + 1) * M, :])]
        seg_w += [(wg[slots[s][c]], wu[slots[s][c]], wd[slots[s][c]])
                  for s in range(4)]
        wgx = np.stack([_pack_gu(g) for g, _, _ in seg_w])
        wux = np.stack([_pack_gu(u) for _, u, _ in seg_w])
        wdx = np.stack([d.reshape(MT, 128, D) for _, _, d in seg_w])

        in_maps.append({
            "xt": xt.reshape(KT, 128, capsum),
            "wg": wgx.astype(NP_DT),
            "wu": wux.astype(NP_DT),
            "wd": wdx.astype(NP_DT),
        })

    t2 = time.time()
    res = run_bass_kernel_spmd(nc, in_maps, core_ids=list(range(N_CORES)))
    t3 = time.time()
    if os.environ.get("BASSMOE_VERBOSE"):
        print(f"[kernel] program build {t1 - t0:.2f}s  pack {t2 - t1:.2f}s  "
              f"device run {t3 - t2:.2f}s", file=sys.stderr)
    outs = res.results

    # slot tensors -> per-core [tok, D] blocks
    def unpack(arr):                                    # [KT, 128, w] -> [w, D]
        return arr.transpose(2, 0, 1).reshape(arr.shape[2], D).astype(np.float32)

    out = np.zeros((T, D), np.float32)
    ytk = np.zeros((T, K, D), np.float32)
    for c in range(N_CORES):
        quarter = c % 4
        ysh = unpack(outs[c]["yr0"])                    # shared-half quarter
        out[quarter * SH_CAP:(quarter + 1) * SH_CAP] += ysh
        y1 = unpack(outs[c]["yr1"])
        y2 = unpack(outs[c]["yr2"])
        lofs = (0, 0, caps[1], 0, caps[3])
        ys = (None, y1, y1, y2, y2)
        for s in range(4):
            e = slots[s][c]
            a = assigns[e]
            if len(a):
                ytk[a // K, a % K] = ys[s + 1][lofs[s + 1]: lofs[s + 1] + len(a)]
    out += (top[:, :, None].astype(np.float32) * ytk).sum(axis=1)
    return out.astype(np.float32)


# revision 31
# speedup vs baseline: 1.0029x; 1.0022x over previous
"""DeepSeek-V3-style MoE layer on 8 Trainium2 NeuronCores.

Strategy (uniform expert-parallel, shared expert folded into routed path):
  - Router (sigmoid over rand_logits, top-4, capacity drop) runs on host:
    it is O(T*E) index math that determines the dispatch, i.e. the sharding.
  - The shared expert (MS = 2816 = 2 x 1408) is exactly two standard-shaped
    experts (D -> M SwiGLU -> D). Each half is token-split 4 ways: cores 0-3
    run half 0, cores 4-7 run half 1, each over a 512-token quarter. This
    removes the 352->384 intermediate padding the sliced layout needed.
  - The 32 routed experts are placed one per (core, segment) cell on a
    4-segment grid; segment capacities are the max routed load in each
    sorted octile (SPMD: every core runs the identical instruction stream).
  - y is written back as [d-tile, 128, tok] fp16 (no on-chip transpose);
    the host transposes, applies routing weights, and scatter-adds.

All matmuls run on the tensor engine with fp16 operands (fp32 PSUM).
"""

import functools
import os
import sys
import time

import numpy as np

for _p in ('/opt/trn_rl_repo', '/root/.axon_site/_ro/trn_rl_repo'):
    if os.path.isdir(_p) and _p not in sys.path:
        sys.path.insert(0, _p)

import concourse.bass as bass  # noqa: F401  (AP helpers)
import concourse.tile as tile
from concourse import bacc, mybir
from concourse.bass_utils import run_bass_kernel_spmd

# ---- problem config (hardcoded from spec) ----
T = 2048
D = 2048          # hidden
M = 1408          # expert intermediate
E = 32            # experts
K = 4             # top_k
CAP = 512         # per-expert capacity
ROUTE_SCALE = 2.5
N_CORES = 8
NSEG = 5          # 1 shared-half segment + 4 routed segments
KT = D // 128     # 16 contraction tiles over hidden
MT = M // 128     # 11 intermediate tiles
SH_CAP = T // 4   # 512 tokens per shared-half quarter

DT, NP_DT = mybir.dt.float16, np.float16
F32 = mybir.dt.float32
SILU = mybir.ActivationFunctionType.Silu


# --------------------------------------------------------------------------
# host-side routing
# --------------------------------------------------------------------------

def _route(rand_logits, expert_bias):
    scores = (1.0 / (1.0 + np.exp(-rand_logits.astype(np.float32)))).astype(np.float32)
    biased = scores + expert_bias[None, :]
    idx = np.argsort(-biased, axis=1, kind="stable")[:, :K]          # [T, K]
    top = np.take_along_axis(scores, idx, axis=1)
    top = top / (top.sum(-1, keepdims=True) + 1e-20) * ROUTE_SCALE   # [T, K]

    flat_e = idx.reshape(-1)
    order = np.argsort(flat_e, kind="stable")                        # assignment ids by expert
    counts = np.bincount(flat_e, minlength=E)
    kept = np.minimum(counts, CAP)
    starts = np.concatenate([[0], np.cumsum(counts)])[:E]
    assigns = [order[starts[e]: starts[e] + kept[e]] for e in range(E)]
    return top, assigns, kept


def _placement(kept):
    """Experts -> (segment, core) grid; segment cap = max load in its octile."""
    rank = np.argsort(-kept, kind="stable")
    slots = rank.reshape(4, N_CORES)                 # routed segment s, core c
    caps = (SH_CAP,) + tuple(int(kept[slots[s][0]]) for s in range(4))
    return slots, caps


# --------------------------------------------------------------------------
# device program
# --------------------------------------------------------------------------

# slot = group of segments sharing one ht/y tensor (keeps DMA runs >= 512B)
SLOT_SEGS = ([0], [1, 2], [3, 4])


@functools.lru_cache(maxsize=4)
def _program(caps):
    capsum = sum(caps)
    offs = [0]
    for c in caps:
        offs.append(offs[-1] + c)

    nc = bacc.Bacc("TRN2", target_bir_lowering=False, debug=False,
                   num_devices=N_CORES)
    ap = {}
    ap["xt"] = nc.dram_tensor("xt", [KT, 128, capsum], DT, kind="ExternalInput").ap()
    ap["wg"] = nc.dram_tensor("wg", [NSEG, MT, 128, KT * 128], DT, kind="ExternalInput").ap()
    ap["wu"] = nc.dram_tensor("wu", [NSEG, MT, 128, KT * 128], DT, kind="ExternalInput").ap()
    ap["wd"] = nc.dram_tensor("wd", [NSEG, MT, 128, D], DT, kind="ExternalInput").ap()
    for si, segs in enumerate(SLOT_SEGS):
        w = sum(caps[s] for s in segs)
        ap[f"yr{si}"] = nc.dram_tensor(f"yr{si}", [KT, 128, w], DT,
                                       kind="ExternalOutput").ap()


    with tile.TileContext(nc) as tc:
        with tc.tile_pool(name="xtp", bufs=1) as xtp, \
             tc.tile_pool(name="wp", bufs=8) as wp, \
             tc.tile_pool(name="hp", bufs=2) as hp, \
             tc.tile_pool(name="wdp", bufs=6) as wdp, \
             tc.tile_pool(name="actp", bufs=3) as actp, \
             tc.tile_pool(name="obp", bufs=4) as obp, \
             tc.tile_pool(name="psgu", bufs=6, space="PSUM") as psgu, \
             tc.tile_pool(name="psy", bufs=2, space="PSUM") as psy:

            xt_sb = xtp.tile([128, KT, capsum], DT, name="xt_sb")

            # PE p-state warmup: the tensor engine idles ~4us waiting for the
            # first DMAs and would then ramp 0.65->1.2->2.4GHz on real work —
            # and with the packed cold-start DMA stream, PE (not DMA) binds
            # right after crossover, so ramp time there is critical-path.
            # Burn the idle window on throwaway matmuls over a memset tile so
            # the clock is already at max when the first real operand lands.
            wz = obp.tile([128, 64], DT, name="warmz", tag="warmz")
            nc.vector.memset(wz[:], 0.0)
            psw = psy.tile([64, 64], F32, name="ps_warm", tag="psy")
            N_WARM = 64
            for i in range(N_WARM):
                nc.tensor.matmul(psw[:], wz[:], wz[:],
                                 start=(i == 0), stop=(i == N_WARM - 1))

            for si, segs in enumerate(SLOT_SEGS):
                soff = offs[segs[0]]                      # global col offset
                scap = sum(caps[s] for s in segs)
                # local (offset, cap) of each segment within the slot
                lseg = []
                o = 0
                for s in segs:
                    lseg.append((s, o, caps[s]))
                    o += caps[s]

                ht = hp.tile([128, MT, scap], DT, name="ht", tag="ht")

                def gu_mm(psg, psu, wg_sb, wu_sb, s, c0, c1):
                    rhs = xt_sb[:, :, offs[s] + c0: offs[s] + c1]
                    for t in range(KT):
                        nc.tensor.matmul(psg[:], wg_sb[:, t * 128:(t + 1) * 128],
                                         rhs[:, t, :], start=(t == 0), stop=(t == KT - 1))
                        nc.tensor.matmul(psu[:], wu_sb[:, t * 128:(t + 1) * 128],
                                         rhs[:, t, :], start=(t == 0), stop=(t == KT - 1))

                def act_mul(psg, psu, c, m, lo):
                    sact = actp.tile([128, c], F32, name="sact", tag="act")
                    nc.scalar.activation(sact[:], psg[:], SILU)
                    nc.vector.tensor_mul(ht[:, m, lo:lo + c], sact[:], psu[:])

                def xtld(t0, t1, c0, c1):
                    nc.sync.dma_start(
                        xt_sb[:, t0:t1, c0:c1],
                        ap["xt"][t0:t1].transpose([1, 0, 2])[:, :, c0:c1])

                for m in range(MT):
                    wpairs = []
                    for s, lo, c in lseg:
                        wg_sb = wp.tile([128, KT * 128], DT, name="wg_sb", tag="w")
                        wu_sb = wp.tile([128, KT * 128], DT, name="wu_sb", tag="w")
                        if si == 0 and m == 0:
                            # cold start: the head's critical path is the DMA
                            # engine itself, so every descriptor carries >=
                            # 625ns of transfer (the HWDGE cadence) — no dead
                            # time — and chunks are issued in need order
                            nc.sync.dma_start(wg_sb[:, :1024], ap["wg"][s, m, :, :1024])
                            xtld(0, 2, 0, SH_CAP)
                            nc.sync.dma_start(wu_sb[:, :1024], ap["wu"][s, m, :, :1024])
                            xtld(2, 4, 0, SH_CAP)
                            nc.sync.dma_start(wg_sb[:, 1024:], ap["wg"][s, m, :, 1024:])
                            nc.sync.dma_start(wu_sb[:, 1024:], ap["wu"][s, m, :, 1024:])
                            xtld(4, 6, 0, SH_CAP)
                            xtld(6, 8, 0, SH_CAP)
                            xtld(8, 10, 0, SH_CAP)
                            xtld(10, 12, 0, SH_CAP)
                            xtld(12, 14, 0, SH_CAP)
                            xtld(14, 16, 0, SH_CAP)
                        else:
                            # stream gate/up halves interleaved so the k-loop
                            # can start before the full m-tile lands
                            nc.sync.dma_start(wg_sb[:, :1024], ap["wg"][s, m, :, :1024])
                            nc.sync.dma_start(wu_sb[:, :1024], ap["wu"][s, m, :, :1024])
                            nc.sync.dma_start(wg_sb[:, 1024:], ap["wg"][s, m, :, 1024:])
                            nc.sync.dma_start(wu_sb[:, 1024:], ap["wu"][s, m, :, 1024:])
                        wpairs.append((wg_sb, wu_sb))
                    if si == 0 and 2 <= m < 10:
                        # backfill routed token columns (needed from slot 1 on)
                        for t in range(2 * (m - 2), 2 * (m - 1)):
                            nc.sync.dma_start(xt_sb[:, t, SH_CAP:],
                                              ap["xt"][t][:, SH_CAP:])

                    for (s, lo, c), (wg_sb, wu_sb) in zip(lseg, wpairs):
                        psg = psgu.tile([128, c], F32, name="psg", tag="psgu")
                        psu = psgu.tile([128, c], F32, name="psu", tag="psgu")
                        gu_mm(psg, psu, wg_sb, wu_sb, s, 0, c)
                        act_mul(psg, psu, c, m, lo)

                # down-projection: out stays [d-part, tok]; host re-layouts
                for g in range(4):
                    wds = []
                    for s, lo, c in lseg:
                        wd_g = wdp.tile([128, MT, 512], DT, name="wd_g", tag="wd")
                        nc.sync.dma_start(
                            wd_g[:],
                            ap["wd"][s].transpose([1, 0, 2])[:, :, g * 512:(g + 1) * 512])
                        wds.append(wd_g)
                    for k in range(4):
                        ob = obp.tile([128, scap], DT, name="ob", tag="ob")
                        for (s, lo, c), wd_g in zip(lseg, wds):
                            ps = psy.tile([128, c], F32, name="ps_y", tag="psy")
                            for m in range(MT):
                                nc.tensor.matmul(ps[:], wd_g[:, m, k * 128:(k + 1) * 128],
                                                 ht[:, m, lo:lo + c],
                                                 start=(m == 0), stop=(m == MT - 1))
                            nc.vector.tensor_copy(ob[:, lo:lo + c], ps[:])
                        nc.sync.dma_start(ap[f"yr{si}"][g * 4 + k], ob[:])
    nc.compile()
    return nc


# --------------------------------------------------------------------------
# host-side packing + combine
# --------------------------------------------------------------------------

def _pack_gu(w):
    # [D, M] -> [MT, 128(k-part), KT*128] stationary-ready layout
    return np.ascontiguousarray(
        w.reshape(KT, 128, MT, 128).transpose(2, 1, 0, 3).reshape(MT, 128, KT * 128))


def kernel(**inputs):
    x = np.asarray(inputs["x"], np.float32)
    rand_logits = np.asarray(inputs["rand_logits"], np.float32)
    expert_bias = np.asarray(inputs["expert_bias"], np.float32)
    wg = np.asarray(inputs["w_gate"], np.float32)
    wu = np.asarray(inputs["w_up"], np.float32)
    wd = np.asarray(inputs["w_down"], np.float32)
    swg = np.asarray(inputs["sw_gate"], np.float32)
    swu = np.asarray(inputs["sw_up"], np.float32)
    swd = np.asarray(inputs["sw_down"], np.float32)

    top, assigns, kept = _route(rand_logits, expert_bias)
    slots, caps = _placement(kept)
    capsum = sum(caps)
    offs = np.concatenate([[0], np.cumsum(caps)]).astype(int)

    global _last_caps
    _last_caps = caps
    t0 = time.time()
    nc = _program(caps)
    t1 = time.time()

    xT = np.ascontiguousarray(x.T.astype(NP_DT))                    # [D, T]

    in_maps = []
    for c in range(N_CORES):
        half, quarter = c // 4, c % 4
        xt = np.zeros((D, capsum), NP_DT)
        xt[:, :SH_CAP] = xT[:, quarter * SH_CAP:(quarter + 1) * SH_CAP]
        for s in range(4):
            e = slots[s][c]
            tok = assigns[e] // K
            if len(tok):
                xt[:, offs[s + 1]: offs[s + 1] + len(tok)] = xT[:, tok]

        seg_w = [(swg[:, half * M:(half + 1) * M],
                  swu[:, half * M:(half + 1) * M],
                  swd[half * M:(half + 1) * M, :])]
        seg_w += [(wg[slots[s][c]], wu[slots[s][c]], wd[slots[s][c]])
                  for s in range(4)]
        wgx = np.stack([_pack_gu(g) for g, _, _ in seg_w])
        wux = np.stack([_pack_gu(u) for _, u, _ in seg_w])
        wdx = np.stack([d.reshape(MT, 128, D) for _, _, d in seg_w])

        in_maps.append({
            "xt": xt.reshape(KT, 128, capsum),
            "wg": wgx.astype(NP_DT),
            "wu": wux.astype(NP_DT),
            "wd": wdx.astype(NP_DT),
        })

    t2 = time.time()
    res = run_bass_kernel_spmd(nc, in_maps, core_ids=list(range(N_CORES)))
    t3 = time.time()
    if os.environ.get("BASSMOE_VERBOSE"):
        print(f"[kernel] program build {t1 - t0:.2f}s  pack {t2 - t1:.2f}s  "
              f"device run {t3 - t2:.2f}s", file=sys.stderr)
    outs = res.results

    # slot tensors -> per-core [tok, D] blocks
    def unpack(arr):                                    # [KT, 128, w] -> [w, D]
        return arr.transpose(2, 0, 1).reshape(arr.shape[2], D).astype(np.float32)

    out = np.zeros((T, D), np.float32)
    ytk = np.zeros((T, K, D), np.float32)
    for c in range(N_CORES):
        quarter = c % 4
        ysh = unpack(outs[c]["yr0"])                    # shared-half quarter
        out[quarter * SH_CAP:(quarter + 1) * SH_CAP] += ysh
        y1 = unpack(outs[c]["yr1"])
        y2 = unpack(outs[c]["yr2"])
        lofs = (0, 0, caps[1], 0, caps[3])
        ys = (None, y1, y1, y2, y2)
        for s in range(4):
            e = slots[s][c]
            a = assigns[e]
            if len(a):
                ytk[a // K, a % K] = ys[s + 1][lofs[s + 1]: lofs[s + 1] + len(a)]
    out += (top[:, :, None].astype(np.float32) * ytk).sum(axis=1)
    return out.astype(np.float32)
